# revision 25
# baseline (speedup 1.0000x reference)
"""Causal self-attention (B=2, S=2048, D=1024, H=16) on 8 trn2 NeuronCores.

Sharding: core c handles batch b = c // 4 and head-group g = c % 4 (4 heads,
256 feature columns).  QKV weights are column-sharded, the output projection
is row-sharded (Megatron style); the host sums the 4 bf16 partial outputs
per batch in f32 and adds the (wv_b @ wo_w + wo_b) correction vector.

Device-side layout (per core), bf16 matmul operands, fp32 psum accumulate:
  xT        [128, 8, 2048]   x[b].T, host pre-transposed (d on partitions)
  qT/kT     2 x [128, 2048]  per fs half: rows = local features (2 heads)
  v2        [128, 64, 128]   block st*4+h = [64 v cols | 64 ones cols]; the
                             ones halves are memset once at start
  logits^T  psum [128, 1024] two k-tiles of one q-block; exp'd on ACT
  av^T      psum [128, 512]  rows 0-63 = unnormalized out^T, rows 64-127 =
                             softmax denominator replicated 64x (the av
                             matmul's lhsT block is [v|ones] so P=128 and the
                             denominator lands pre-broadcast -> normalization
                             is copy+recip+mul on DVE; the copy is required
                             because custom-DVE ops need SBUF inputs on HW)
  avT       2 x [128, 2048]  normalized attention output, transposed
  out       [2048, 1024]     partial (pre-reduction) output, bf16

Causal diagonal trimming: for diagonal k-tiles only the unmasked q columns
are computed (lg + av matmuls and the exp), cutting ~8% of PE columns and
~12% of ACT work; ragged units exp each k-tile range separately so no
uninitialized psum is read.

Attention k-tiles are processed in ASCENDING order per head and pipelined
one pair ahead (av of pair j is emitted after lg+exp of pair j+1).  All
non-attention PE work (qk/v projection groups, output-projection tiles)
lives in a FIFO "filler" queue; a coarse 2-engine clock model drains just
enough filler before each av (and each lg bank-WAR point) that the PE never
idles waiting on the activation engine.  Output-projection items are also
retired eagerly (one per attention unit once available) so their DMAs
overlap compute instead of forming a drain tail; the final head's
normalization is emitted in 128-column pieces for the same reason.

PSUM budget (8 banks): "small" [128,512] x2 (projection + out-proj tiles),
"av" [128,512] x2 (attention accumulators), "lg" [128,1024] x2 = 4 banks.
"""

import os

import ml_dtypes
import numpy as np

import concourse.bass as bass
import concourse.mybir as mybir
import concourse.tile as tile
from concourse import bacc
from concourse.bass_utils import run_bass_kernel_spmd

F32 = mybir.dt.float32
F32R = mybir.dt.float32r
BF16 = mybir.dt.bfloat16
AF = mybir.ActivationFunctionType

B, S, D = 2, 2048, 1024
H, DH = 16, 64          # heads, head depth
G = 4                   # head groups (cores per batch)
HPG = H // G            # heads per group = 4
F = HPG * DH            # local feature columns = 256
KC = D // 128           # contraction chunks = 8
ST = S // 128           # seq tiles of 128 = 16
QB = S // 512           # q blocks of 512 = 4
SCALE = 1.0 / float(np.sqrt(DH))


def _build(allones: bool):
    nc = bacc.Bacc("TRN2", target_bir_lowering=False, debug=False)

    xT_d = nc.dram_tensor("xT", [4, 128, KC, 512], BF16, kind="ExternalInput")
    wq_d = nc.dram_tensor("wq", [128, KC, F], BF16, kind="ExternalInput")
    wk_d = nc.dram_tensor("wk", [128, KC, F], BF16, kind="ExternalInput")
    wv_d = nc.dram_tensor("wv", [128, KC, F], BF16, kind="ExternalInput")
    wo_d = nc.dram_tensor("wo", [128, 2, D], BF16, kind="ExternalInput")
    bq_d = nc.dram_tensor("bq", [128, 2], F32, kind="ExternalInput")
    bk_d = nc.dram_tensor("bk", [128, 2], F32, kind="ExternalInput")
    out_d = nc.dram_tensor("out", [S, D], BF16, kind="ExternalOutput")
    dbg = bool(int(os.environ.get("KDBG", "0")))
    if dbg:
        dbg_v2 = nc.dram_tensor("dbg_v2", [128, ST * HPG, 128], BF16, kind="ExternalOutput")
        dbg_avT = nc.dram_tensor("dbg_avT", [2, 128, S], BF16, kind="ExternalOutput")
    if not allones:
        pad_d = nc.dram_tensor("pad", [1, S], BF16, kind="ExternalInput")

    with tile.TileContext(nc) as tc:
        with (
            tc.tile_pool(name="singles", bufs=1) as singles,
            tc.tile_pool(name="expp", bufs=6) as expp,
            tc.tile_pool(name="recipp", bufs=4) as recipp,
            tc.tile_pool(name="outsbp", bufs=3) as outsbp,
            tc.tile_pool(name="psum", bufs=2, space="PSUM") as psum,
        ):
            xT = singles.tile([128, 4, KC, 512], BF16, tag="xT")
            wq = singles.tile([128, KC, F], BF16, tag="wq")
            wk = singles.tile([128, KC, F], BF16, tag="wk")
            wv = singles.tile([128, KC, F], BF16, tag="wv")
            wo = singles.tile([128, 2, D], BF16, tag="wo")
            qT = [singles.tile([128, S], BF16, tag=f"qT{i}", name=f"qT{i}") for i in range(2)]
            kT = [singles.tile([128, S], BF16, tag=f"kT{i}", name=f"kT{i}") for i in range(2)]
            v2 = singles.tile([128, ST * HPG, 128], BF16, tag="v2")
            avT = [singles.tile([128, S], BF16, tag=f"avT{i}", name=f"avT{i}") for i in range(2)]
            bq = singles.tile([128, 2], F32, tag="bq")
            bk = singles.tile([128, 2], F32, tag="bk")

            # --- input DMAs, two stages.  The DGE runs all queued transfers
            # concurrently (bandwidth-shared), so to give the critical first
            # tiles full bandwidth the remaining loads are gated behind an
            # xT0-dependent touch op (WAW on their destination tiles).  wq and
            # xT0 are split into kc halves (separate DMAs) so the first
            # projection matmuls can start after the first halves land.
            KH = KC // 2
            nc.sync.dma_start(out=wq[:, 0:KH], in_=wq_d.ap()[:, 0:KH])
            nc.sync.dma_start(out=xT[:, 0, 0:KH], in_=xT_d.ap()[0][:, 0:KH])
            nc.sync.dma_start(out=wq[:, KH:KC], in_=wq_d.ap()[:, KH:KC])
            nc.sync.dma_start(out=xT[:, 0, KH:KC], in_=xT_d.ap()[0][:, KH:KC])
            nc.sync.dma_start(out=wk, in_=wk_d.ap())
            nc.sync.dma_start(out=wv, in_=wv_d.ap())
            nc.sync.dma_start(out=bq, in_=bq_d.ap())
            nc.sync.dma_start(out=bk, in_=bk_d.ap())
            for sch in range(1, 4):
                nc.vector.tensor_copy(xT[:, sch, 0, 0:1], xT[:, 0, KC - 1, 0:1])
            nc.vector.tensor_copy(wo[:, 0, 0:1], xT[:, 0, KC - 1, 0:1])
            for sch in range(1, 4):
                nc.sync.dma_start(out=xT[:, sch], in_=xT_d.ap()[sch])
            nc.sync.dma_start(out=wo, in_=wo_d.ap())
            nc.gpsimd.memset(v2[:, :, 64:128], 1.0)
            if not allones:
                pad_sb = singles.tile([1, S], BF16, tag="pad")
                ones512 = singles.tile([1, 512], BF16, tag="ones512")
                nc.sync.dma_start(out=pad_sb, in_=pad_d.ap())
                nc.vector.memset(ones512, 1.0)

            # ---------------- filler queue + 2-engine clock model -----------
            # All non-attention PE work (qk/v projections, output projection
            # tiles) lives in a FIFO of small items.  A coarse model of the
            # PE and ACT clocks decides, at each point where the attention
            # stream is about to wait on an exp (or an lg psum-bank WAR),
            # how many filler items to drain so the PE never idles.
            PE_COL = 1.0 / 2.4   # ns per streamed matmul column at full clock
            MM_OVH = 45.0        # per-matmul fixed overhead (empirical)
            ACT_COL = 1.0 / 1.2  # ns per column on the activation engine
            ACT_OVH = 190.0      # per-activation overhead incl access latency
            SEM = 100.0          # cross-engine semaphore propagation

            clk = {"pe": 0.0, "act": 0.0}
            fillq = []
            drained = set()

            def pe_op(dur, ready=0.0):
                clk["pe"] = max(clk["pe"], ready) + dur

            def qk_item(which, sch, fs):
                w_sb, b_sb, dst = (wq, bq, qT) if which == "q" else (wk, bk, kT)

                def fn():
                    ssl = bass.ds(sch * 512, 512)
                    fsl = bass.ds(fs * 128, 128)
                    ps = psum.tile([128, 512], F32, tag="small", bufs=2, name="ps_qk")
                    for kc in range(KC):
                        nc.tensor.matmul(
                            ps,
                            lhsT=w_sb[:, kc, fsl],
                            rhs=xT[:, sch, kc, :],
                            start=(kc == 0),
                            stop=(kc == KC - 1),
                        )
                    nc.vector.tensor_scalar_add(dst[fs][:, ssl], ps, b_sb[:, fs : fs + 1])

                return (8 * 512 * PE_COL + 8 * MM_OVH, ("qk", which, sch, fs), fn)

            def v_item(st):
                def fn():
                    vps = psum.tile([128, F], F32, tag="small", bufs=2, name="ps_v")
                    for kc in range(KC):
                        nc.tensor.matmul(
                            vps,
                            lhsT=xT[:, st // 4, kc, bass.ds((st % 4) * 128, 128)],
                            rhs=wv[:, kc, :],
                            start=(kc == 0),
                            stop=(kc == KC - 1),
                        )
                    # strided view of vps as [128, 4 heads, 64] so one copy
                    # scatters the four heads' v columns into their blocks
                    vps3 = bass.AP(vps.tensor, vps.offset, [vps.ap[0], [DH, HPG], [1, DH]])
                    nc.vector.tensor_copy(v2[:, st * HPG : (st + 1) * HPG, 0:64], vps3)

                return (8 * 256 * PE_COL + 8 * MM_OVH, ("v", st), fn)

            obs = {}

            def oproj_item(st, eh):
                def fn():
                    if st not in obs:
                        obs[st] = outsbp.tile([128, D], BF16, tag="ob", name="ob")
                    ob = obs[st]
                    op = psum.tile([128, 512], F32, tag="small", bufs=2, name="ps_op")
                    for fs in range(2):
                        nc.tensor.matmul(
                            op,
                            lhsT=avT[fs][:, bass.ds(st * 128, 128)],
                            rhs=wo[:, fs, bass.ds(eh * 512, 512)],
                            start=(fs == 0),
                            stop=(fs == 1),
                        )
                    nc.vector.tensor_copy(ob[:, bass.ds(eh * 512, 512)], op)
                    if eh == 1:
                        # one DMA for the whole [128, 1024] row block: the
                        # DRAM destination is a contiguous 256 KB region, so
                        # descriptors coalesce (vs ~1 KB strided rows when
                        # writing eh halves separately, which throttles the
                        # writeback to a fraction of HBM bandwidth)
                        eng = nc.sync if st % 2 == 0 else nc.scalar
                        eng.dma_start(
                            out=out_d.ap()[bass.ds(st * 128, 128), :],
                            in_=ob,
                        )
                        del obs[st]

                return (2 * 512 * PE_COL + 2 * MM_OVH, ("oproj", st, eh), fn)

            def drain_one():
                pe_ns, key, fn = fillq.pop(0)
                fn()
                pe_op(pe_ns)
                drained.add(key)

            def drain_until_pe(t):
                while clk["pe"] < t and fillq:
                    drain_one()

            def drain_through(key):
                while key not in drained:
                    assert fillq, f"filler queue exhausted before {key}"
                    drain_one()

            # enqueue all projection work; oproj items are appended at each
            # q-block flush once their avT inputs exist
            for sch in range(4):
                fillq.append(qk_item("q", sch, 0))
                fillq.append(qk_item("k", sch, 0))
                for st in range(4 * sch, 4 * sch + 4):
                    fillq.append(v_item(st))
                fillq.append(qk_item("q", sch, 1))
                fillq.append(qk_item("k", sch, 1))

            # attention is software-pipelined one k-tile pair ahead: each
            # pair's av matmuls are emitted only after the NEXT pair's lg
            # matmuls + exp are queued, so the exp latency is covered by PE
            # work; filler items are drained whenever the clock model says
            # the PE would otherwise reach an av before its exp finishes.
            avs = {}
            pending = []  # [(qb, h, kts, ex, exp_done_ns)]

            # causal diagonal trimming: for a diagonal k-tile kt (kt >= 4*qb)
            # only q columns >= kt*128 are unmasked, i.e. block-local columns
            # f >= f0 = (kt-4*qb)*128.  The lg matmul, exp and av matmul all
            # skip the fully-masked [0, f0) column range; the av accumulation
            # is safe because kt=0 (full width, start=True) zero-fills the
            # whole psum bank and trimmed kts simply don't touch [0, f0).
            def _f0(qb, kt):
                return max(0, (kt - 4 * qb) * 128)

            exp_hist = []  # exp-chain completion times, for the lg-bank WAR

            def emit_unit_lgexp(qb, h, kts):
                fs, hh = h // 2, h % 2
                hsl = bass.ds(hh * 64, 64)
                lg = psum.tile([128, 1024], F32, tag="lg", name="ps_lg")
                f0s = [_f0(qb, kt) for kt in kts]
                for i, (kt, f0) in enumerate(zip(kts, f0s)):
                    osl = bass.ds(i * 512 + f0, 512 - f0)
                    qsl = bass.ds(qb * 512 + f0, 512 - f0)
                    if not allones:
                        nc.tensor.matmul(
                            lg[:, osl],
                            lhsT=pad_sb[:, bass.ds(kt * 128, 128)],
                            rhs=ones512[:, 0 : 512 - f0],
                            start=True,
                            stop=False,
                        )
                        pe_op((512 - f0) * PE_COL + MM_OVH)
                    nc.tensor.matmul(
                        lg[:, osl],
                        lhsT=kT[fs][hsl, bass.ds(kt * 128, 128)],
                        rhs=qT[fs][hsl, qsl],
                        start=allones,
                        stop=True,
                    )
                    pe_op((512 - f0) * PE_COL + MM_OVH)
                ex = expp.tile([128, 1024], BF16, tag="ex", name="ex")
                ready = clk["pe"] + SEM
                if all(f0 == 0 for f0 in f0s):
                    # one contiguous exp over the whole unit
                    nc.scalar.activation(
                        ex[:, 0 : 512 * len(kts)],
                        lg[:, 0 : 512 * len(kts)],
                        AF.Exp,
                        scale=SCALE,
                    )
                    clk["act"] = max(clk["act"], ready) + 512 * len(kts) * ACT_COL + ACT_OVH
                else:
                    # ragged trims: exp each kt's written range separately so
                    # no uninitialized psum gap is ever read
                    for i, f0 in enumerate(f0s):
                        rsl = bass.ds(i * 512 + f0, 512 - f0)
                        nc.scalar.activation(ex[:, rsl], lg[:, rsl], AF.Exp, scale=SCALE)
                        clk["act"] = max(clk["act"], ready) + (512 - f0) * ACT_COL + ACT_OVH
                done = clk["act"] + SEM
                for i, (kt, f0) in enumerate(zip(kts, f0s)):
                    if kt >= 4 * qb:  # diagonal tile: causal mask
                        w = 512 - f0
                        osl = bass.ds(i * 512 + f0, w)
                        nc.gpsimd.affine_select(
                            out=ex[:, osl],
                            in_=ex[:, osl],
                            compare_op=mybir.AluOpType.is_ge,
                            fill=0.0,
                            base=0,
                            channel_multiplier=-1,
                            pattern=[[1, w]],
                        )
                        done += w * ACT_COL + 150.0  # pool op in the exp->av chain
                exp_hist.append(done)
                return ex, done

            def _emit_one_av(unit):
                qb, h, kts, ex, exp_done = unit
                fs, hh = h // 2, h % 2
                hsl = bass.ds(hh * 64, 64)
                qsl = bass.ds(qb * 512, 512)
                nkt = 4 * qb + 4
                if (qb, h) not in avs:
                    avs[(qb, h)] = psum.tile(
                        [128, 512], F32, tag="av", bufs=2, name="ps_av"
                    )
                av = avs[(qb, h)]
                for i, kt in enumerate(kts):
                    f0 = _f0(qb, kt)
                    nc.tensor.matmul(
                        av[:, bass.ds(f0, 512 - f0)],
                        lhsT=v2[:, kt * HPG + h, :],
                        rhs=ex[:, bass.ds(i * 512 + f0, 512 - f0)],
                        start=(kt == 0),
                        stop=(kt == nkt - 1),
                    )
                    pe_op((512 - f0) * PE_COL + MM_OVH, ready=(exp_done if i == 0 else 0.0))
                if kts[-1] == nkt - 1:  # head complete: normalize
                    # NOTE: custom-DVE ops (reciprocal_approx_fast) must read
                    # SBUF on hardware -- a direct PSUM input passes CoreSim
                    # but returns garbage on HW.  Hence the copy first.
                    den = recipp.tile([64, 512], F32, tag="den", name="den")
                    rf = recipp.tile([64, 512], F32, tag="rf", name="rf")
                    if qb == QB - 1 and h == HPG - 1:
                        # the very last normalize gates the final output
                        # projection tiles: emit it in 128-column pieces so
                        # oproj(st) can start as soon as its piece lands
                        for p in range(4):
                            psl = bass.ds(p * 128, 128)
                            qpl = bass.ds(qb * 512 + p * 128, 128)
                            nc.vector.tensor_copy(den[:, psl], av[64:128, psl])
                            nc.vector.reciprocal_approx_fast(rf[:, psl], den[:, psl])
                            nc.vector.tensor_mul(
                                avT[fs][hsl, qpl], av[0:64, psl], rf[:, psl]
                            )
                    else:
                        nc.vector.tensor_copy(den, av[64:128, :])
                        nc.vector.reciprocal_approx_fast(rf, den)
                        nc.vector.tensor_mul(avT[fs][hsl, qsl], av[0:64, :], rf)
                    del avs[(qb, h)]

            def flush_pending():
                while pending:
                    u = pending.pop(0)
                    drain_until_pe(u[4])
                    _emit_one_av(u)

            def emit_attention_head(qb, h):
                nkt = 4 * qb + 4
                for j in range(nkt // 2):
                    kts = [2 * j, 2 * j + 1]
                    # lg psum-bank WAR: the lg tag has 2 tiles; this unit's lg
                    # bank is free only once the exp two units back has read it
                    if len(exp_hist) >= 2:
                        drain_until_pe(exp_hist[-2] - SEM)
                    ex, exp_done = emit_unit_lgexp(qb, h, kts)
                    prev = pending[:]
                    del pending[:]
                    pending.append((qb, h, kts, ex, exp_done))
                    for u in prev:
                        drain_until_pe(u[4])
                        _emit_one_av(u)

            for qb in range(QB):
                for h in range(HPG):
                    if h == 0:
                        drain_through(("v", 4 * qb + 3))
                        drain_through(("qk", "k", qb, 0))
                    if h == 2:
                        drain_through(("qk", "k", qb, 1))
                    with nc.named_scope(f"attn{qb}h{h}"):
                        emit_attention_head(qb, h)
                flush_pending()
                for st in range(4 * qb, 4 * qb + 4):
                    fillq.append(oproj_item(st, 0))
                    fillq.append(oproj_item(st, 1))
            with nc.named_scope("tailproj"):
                while fillq:
                    drain_one()

            if dbg:
                nc.sync.dma_start(out=dbg_v2.ap(), in_=v2)
                for i in range(2):
                    nc.sync.dma_start(out=dbg_avT.ap()[i], in_=avT[i])

    nc.compile()
    return nc


_CACHE: dict = {}


def kernel(
    x,
    padding_mask,
    wq_w,
    wq_b,
    wk_w,
    wk_b,
    wv_w,
    wv_b,
    wo_w,
    wo_b,
    **trace_kwargs,
):
    x = np.asarray(x, dtype=np.float32)
    padding_mask = np.asarray(padding_mask, dtype=np.float32)
    wq_w = np.asarray(wq_w, dtype=np.float32)
    wk_w = np.asarray(wk_w, dtype=np.float32)
    wv_w = np.asarray(wv_w, dtype=np.float32)
    wo_w = np.asarray(wo_w, dtype=np.float32)
    wq_b = np.asarray(wq_b, dtype=np.float32)
    wk_b = np.asarray(wk_b, dtype=np.float32)
    wv_b = np.asarray(wv_b, dtype=np.float32)
    wo_b = np.asarray(wo_b, dtype=np.float32)

    allones = bool(np.all(padding_mask == 1.0))
    if allones not in _CACHE:
        _CACHE[allones] = _build(allones)
    nc = _CACHE[allones]

    bf = ml_dtypes.bfloat16
    in_maps = []
    for c in range(8):
        b, g = c // 4, c % 4
        fsl = slice(g * F, (g + 1) * F)
        xTb = x[b].T.astype(bf)  # (1024, 2048)
        m = {
            # [4 sch, 128 p, KC, 512]: xT[d, s] with d = kc*128 + p
            "xT": np.ascontiguousarray(
                xTb.reshape(KC, 128, 4, 512).transpose(2, 1, 0, 3)
            ),
            "wq": np.ascontiguousarray(
                wq_w[:, fsl].astype(bf).reshape(KC, 128, F).transpose(1, 0, 2)
            ),
            "wk": np.ascontiguousarray(
                wk_w[:, fsl].astype(bf).reshape(KC, 128, F).transpose(1, 0, 2)
            ),
            "wv": np.ascontiguousarray(
                wv_w[:, fsl].astype(bf).reshape(KC, 128, F).transpose(1, 0, 2)
            ),
            "wo": np.ascontiguousarray(
                wo_w[fsl, :].astype(bf).reshape(2, 128, D).transpose(1, 0, 2)
            ),
            "bq": np.ascontiguousarray(wq_b[fsl].reshape(2, 128).T),
            "bk": np.ascontiguousarray(wk_b[fsl].reshape(2, 128).T),
        }
        if not allones:
            m["pad"] = ((padding_mask[b] - 1.0) * 8e9).reshape(1, S).astype(bf)
        in_maps.append(m)

    res = run_bass_kernel_spmd(nc, in_maps, core_ids=list(range(8)), **trace_kwargs)

    # host-side reduction over head groups (bf16 partials -> f32) + bias
    # correction
    correction = (wv_b @ wo_w + wo_b).astype(np.float32)
    out = np.empty((B, S, D), dtype=np.float32)
    for b in range(B):
        acc = res.results[4 * b]["out"].astype(np.float32)
        for g in range(1, G):
            acc += res.results[4 * b + g]["out"].astype(np.float32)
        out[b] = acc + correction
    kernel._last_results = res
    return out



# revision 28
# speedup vs baseline: 1.0048x; 1.0048x over previous
"""Causal self-attention (B=2, S=2048, D=1024, H=16) on 8 trn2 NeuronCores.

Sharding: core c handles batch b = c // 4 and head-group g = c % 4 (4 heads,
256 feature columns).  QKV weights are column-sharded, the output projection
is row-sharded (Megatron style); the host sums the 4 bf16 partial outputs
per batch in f32 and adds the (wv_b @ wo_w + wo_b) correction vector.

Device-side layout (per core), bf16 matmul operands, fp32 psum accumulate:
  xT        [128, 8, 2048]   x[b].T, host pre-transposed (d on partitions)
  qT/kT     2 x [128, 2048]  per fs half: rows = local features (2 heads)
  v2        [128, 64, 128]   block st*4+h = [64 v cols | 64 ones cols]; the
                             ones halves are memset once at start
  logits^T  psum [128, 1024] two k-tiles of one q-block; exp'd on ACT
  av^T      psum [128, 512]  rows 0-63 = unnormalized out^T, rows 64-127 =
                             softmax denominator replicated 64x (the av
                             matmul's lhsT block is [v|ones] so P=128 and the
                             denominator lands pre-broadcast -> normalization
                             is copy+recip+mul on DVE; the copy is required
                             because custom-DVE ops need SBUF inputs on HW)
  avT       2 x [128, 2048]  normalized attention output, transposed
  out       [2048, 1024]     partial (pre-reduction) output, bf16

Causal diagonal trimming: for diagonal k-tiles only the unmasked q columns
are computed (lg + av matmuls and the exp), cutting ~8% of PE columns and
~12% of ACT work; ragged units exp each k-tile range separately so no
uninitialized psum is read.

Attention k-tiles are processed in ASCENDING order per head and pipelined
one pair ahead (av of pair j is emitted after lg+exp of pair j+1).  All
non-attention PE work (qk/v projection groups, output-projection tiles)
lives in a FIFO "filler" queue; a coarse 2-engine clock model drains just
enough filler before each av (and each lg bank-WAR point) that the PE never
idles waiting on the activation engine.  Output-projection items are also
retired eagerly (one per attention unit once available) so their DMAs
overlap compute instead of forming a drain tail; the final head's
normalization is emitted in 128-column pieces for the same reason.

PSUM budget (8 banks): "small" [128,512] x2 (projection + out-proj tiles),
"av" [128,512] x2 (attention accumulators), "lg" [128,1024] x2 = 4 banks.
"""

import os

import ml_dtypes
import numpy as np

import concourse.bass as bass
import concourse.mybir as mybir
import concourse.tile as tile
from concourse import bacc
from concourse.bass_utils import run_bass_kernel_spmd

F32 = mybir.dt.float32
F32R = mybir.dt.float32r
BF16 = mybir.dt.bfloat16
AF = mybir.ActivationFunctionType

B, S, D = 2, 2048, 1024
H, DH = 16, 64          # heads, head depth
G = 4                   # head groups (cores per batch)
HPG = H // G            # heads per group = 4
F = HPG * DH            # local feature columns = 256
KC = D // 128           # contraction chunks = 8
ST = S // 128           # seq tiles of 128 = 16
QB = S // 512           # q blocks of 512 = 4
SCALE = 1.0 / float(np.sqrt(DH))


def _build(allones: bool):
    nc = bacc.Bacc("TRN2", target_bir_lowering=False, debug=False)

    xT_d = nc.dram_tensor("xT", [4, 128, KC, 512], BF16, kind="ExternalInput")
    wq_d = nc.dram_tensor("wq", [128, KC, F], BF16, kind="ExternalInput")
    wk_d = nc.dram_tensor("wk", [128, KC, F], BF16, kind="ExternalInput")
    wv_d = nc.dram_tensor("wv", [128, KC, F], BF16, kind="ExternalInput")
    wo_d = nc.dram_tensor("wo", [128, 2, D], BF16, kind="ExternalInput")
    bq_d = nc.dram_tensor("bq", [128, 2], F32, kind="ExternalInput")
    bk_d = nc.dram_tensor("bk", [128, 2], F32, kind="ExternalInput")
    out_d = nc.dram_tensor("out", [S, D], BF16, kind="ExternalOutput")
    dbg = bool(int(os.environ.get("KDBG", "0")))
    if dbg:
        dbg_v2 = nc.dram_tensor("dbg_v2", [128, ST * HPG, 128], BF16, kind="ExternalOutput")
        dbg_avT = nc.dram_tensor("dbg_avT", [2, 128, S], BF16, kind="ExternalOutput")
    if not allones:
        pad_d = nc.dram_tensor("pad", [1, S], BF16, kind="ExternalInput")

    with tile.TileContext(nc) as tc:
        with (
            tc.tile_pool(name="singles", bufs=1) as singles,
            tc.tile_pool(name="expp", bufs=6) as expp,
            tc.tile_pool(name="recipp", bufs=4) as recipp,
            tc.tile_pool(name="outsbp", bufs=3) as outsbp,
            tc.tile_pool(name="psum", bufs=2, space="PSUM") as psum,
        ):
            xT = singles.tile([128, 4, KC, 512], BF16, tag="xT")
            wq = singles.tile([128, KC, F], BF16, tag="wq")
            wk = singles.tile([128, KC, F], BF16, tag="wk")
            wv = singles.tile([128, KC, F], BF16, tag="wv")
            wo = singles.tile([128, 2, D], BF16, tag="wo")
            qT = [singles.tile([128, S], BF16, tag=f"qT{i}", name=f"qT{i}") for i in range(2)]
            kT = [singles.tile([128, S], BF16, tag=f"kT{i}", name=f"kT{i}") for i in range(2)]
            v2 = singles.tile([128, ST * HPG, 128], BF16, tag="v2")
            avT = [singles.tile([128, S], BF16, tag=f"avT{i}", name=f"avT{i}") for i in range(2)]
            bq = singles.tile([128, 2], F32, tag="bq")
            bk = singles.tile([128, 2], F32, tag="bk")

            # --- input DMAs, two stages.  The DGE runs all queued transfers
            # concurrently (bandwidth-shared), so to give the critical first
            # tiles full bandwidth the remaining loads are gated behind an
            # xT0-dependent touch op (WAW on their destination tiles).  wq and
            # xT0 are split into kc halves (separate DMAs) so the first
            # projection matmuls can start after the first halves land.
            KH = KC // 2
            nc.sync.dma_start(out=wq[:, 0:KH], in_=wq_d.ap()[:, 0:KH])
            nc.sync.dma_start(out=xT[:, 0, 0:KH], in_=xT_d.ap()[0][:, 0:KH])
            nc.sync.dma_start(out=wq[:, KH:KC], in_=wq_d.ap()[:, KH:KC])
            nc.sync.dma_start(out=xT[:, 0, KH:KC], in_=xT_d.ap()[0][:, KH:KC])
            nc.sync.dma_start(out=wk, in_=wk_d.ap())
            nc.sync.dma_start(out=wv, in_=wv_d.ap())
            nc.sync.dma_start(out=bq, in_=bq_d.ap())
            nc.sync.dma_start(out=bk, in_=bk_d.ap())
            for sch in range(1, 4):
                nc.vector.tensor_copy(xT[:, sch, 0, 0:1], xT[:, 0, KC - 1, 0:1])
            nc.vector.tensor_copy(wo[:, 0, 0:1], xT[:, 0, KC - 1, 0:1])
            for sch in range(1, 4):
                nc.sync.dma_start(out=xT[:, sch], in_=xT_d.ap()[sch])
            nc.sync.dma_start(out=wo, in_=wo_d.ap())
            nc.gpsimd.memset(v2[:, :, 64:128], 1.0)
            if not allones:
                pad_sb = singles.tile([1, S], BF16, tag="pad")
                ones512 = singles.tile([1, 512], BF16, tag="ones512")
                nc.sync.dma_start(out=pad_sb, in_=pad_d.ap())
                nc.vector.memset(ones512, 1.0)

            # ---------------- filler queue + 2-engine clock model -----------
            # All non-attention PE work (qk/v projections, output projection
            # tiles) lives in a FIFO of small items.  A coarse model of the
            # PE and ACT clocks decides, at each point where the attention
            # stream is about to wait on an exp (or an lg psum-bank WAR),
            # how many filler items to drain so the PE never idles.
            PE_COL = 1.0 / 2.4   # ns per streamed matmul column at full clock
            MM_OVH = 45.0        # per-matmul fixed overhead (empirical)
            ACT_COL = 1.0 / 1.2  # ns per column on the activation engine
            ACT_OVH = 190.0      # per-activation overhead incl access latency
            SEM = 100.0          # cross-engine semaphore propagation

            clk = {"pe": 0.0, "act": 0.0}
            fillq = []
            drained = set()

            def pe_op(dur, ready=0.0):
                clk["pe"] = max(clk["pe"], ready) + dur

            def qk_item(which, sch, fs):
                w_sb, b_sb, dst = (wq, bq, qT) if which == "q" else (wk, bk, kT)

                def fn():
                    ssl = bass.ds(sch * 512, 512)
                    fsl = bass.ds(fs * 128, 128)
                    ps = psum.tile([128, 512], F32, tag="small", bufs=2, name="ps_qk")
                    for kc in range(KC):
                        nc.tensor.matmul(
                            ps,
                            lhsT=w_sb[:, kc, fsl],
                            rhs=xT[:, sch, kc, :],
                            start=(kc == 0),
                            stop=(kc == KC - 1),
                        )
                    nc.vector.tensor_scalar_add(dst[fs][:, ssl], ps, b_sb[:, fs : fs + 1])

                return (8 * 512 * PE_COL + 8 * MM_OVH, ("qk", which, sch, fs), fn)

            def v_item(st):
                def fn():
                    vps = psum.tile([128, F], F32, tag="small", bufs=2, name="ps_v")
                    for kc in range(KC):
                        nc.tensor.matmul(
                            vps,
                            lhsT=xT[:, st // 4, kc, bass.ds((st % 4) * 128, 128)],
                            rhs=wv[:, kc, :],
                            start=(kc == 0),
                            stop=(kc == KC - 1),
                        )
                    # strided view of vps as [128, 4 heads, 64] so one copy
                    # scatters the four heads' v columns into their blocks
                    vps3 = bass.AP(vps.tensor, vps.offset, [vps.ap[0], [DH, HPG], [1, DH]])
                    nc.vector.tensor_copy(v2[:, st * HPG : (st + 1) * HPG, 0:64], vps3)

                return (8 * 256 * PE_COL + 8 * MM_OVH, ("v", st), fn)

            obs = {}

            def oproj_item(st, eh):
                def fn():
                    if st not in obs:
                        obs[st] = outsbp.tile([128, D], BF16, tag="ob", name="ob")
                    ob = obs[st]
                    op = psum.tile([128, 512], F32, tag="small", bufs=2, name="ps_op")
                    for fs in range(2):
                        nc.tensor.matmul(
                            op,
                            lhsT=avT[fs][:, bass.ds(st * 128, 128)],
                            rhs=wo[:, fs, bass.ds(eh * 512, 512)],
                            start=(fs == 0),
                            stop=(fs == 1),
                        )
                    nc.vector.tensor_copy(ob[:, bass.ds(eh * 512, 512)], op)
                    if eh == 1:
                        # one DMA for the whole [128, 1024] row block: the
                        # DRAM destination is a contiguous 256 KB region, so
                        # descriptors coalesce (vs ~1 KB strided rows when
                        # writing eh halves separately, which throttles the
                        # writeback to a fraction of HBM bandwidth)
                        eng = nc.sync if st % 2 == 0 else nc.scalar
                        eng.dma_start(
                            out=out_d.ap()[bass.ds(st * 128, 128), :],
                            in_=ob,
                        )
                        del obs[st]

                return (2 * 512 * PE_COL + 2 * MM_OVH, ("oproj", st, eh), fn)

            def drain_one():
                pe_ns, key, fn = fillq.pop(0)
                fn()
                pe_op(pe_ns)
                drained.add(key)

            def drain_until_pe(t):
                while clk["pe"] < t and fillq:
                    drain_one()

            def drain_through(key):
                while key not in drained:
                    assert fillq, f"filler queue exhausted before {key}"
                    drain_one()

            # enqueue all projection work; oproj items are appended at each
            # q-block flush once their avT inputs exist
            for sch in range(4):
                fillq.append(qk_item("q", sch, 0))
                fillq.append(qk_item("k", sch, 0))
                for st in range(4 * sch, 4 * sch + 4):
                    fillq.append(v_item(st))
                fillq.append(qk_item("q", sch, 1))
                fillq.append(qk_item("k", sch, 1))

            # attention is software-pipelined one k-tile pair ahead: each
            # pair's av matmuls are emitted only after the NEXT pair's lg
            # matmuls + exp are queued, so the exp latency is covered by PE
            # work; filler items are drained whenever the clock model says
            # the PE would otherwise reach an av before its exp finishes.
            avs = {}
            pending = []  # [(qb, h, kts, ex, exp_done_ns)]

            # causal diagonal trimming: for a diagonal k-tile kt (kt >= 4*qb)
            # only q columns >= kt*128 are unmasked, i.e. block-local columns
            # f >= f0 = (kt-4*qb)*128.  The lg matmul, exp and av matmul all
            # skip the fully-masked [0, f0) column range; the av accumulation
            # is safe because kt=0 (full width, start=True) zero-fills the
            # whole psum bank and trimmed kts simply don't touch [0, f0).
            def _f0(qb, kt):
                return max(0, (kt - 4 * qb) * 128)

            exp_hist = []  # exp-chain completion times, for the lg-bank WAR

            def emit_unit_lgexp(qb, h, kts):
                fs, hh = h // 2, h % 2
                # finest-grain forced drains: this unit's lg matmuls read
                # qT[fs] for s-chunk qb and kT[fs] for s-chunks <= kts[-1]//4;
                # FIFO order guarantees everything enqueued earlier comes too
                drain_through(("qk", "q", qb, fs))
                if kts[-1] // 4 == qb:
                    drain_through(("qk", "k", qb, fs))
                hsl = bass.ds(hh * 64, 64)
                lg = psum.tile([128, 1024], F32, tag="lg", name="ps_lg")
                f0s = [_f0(qb, kt) for kt in kts]
                for i, (kt, f0) in enumerate(zip(kts, f0s)):
                    osl = bass.ds(i * 512 + f0, 512 - f0)
                    qsl = bass.ds(qb * 512 + f0, 512 - f0)
                    if not allones:
                        nc.tensor.matmul(
                            lg[:, osl],
                            lhsT=pad_sb[:, bass.ds(kt * 128, 128)],
                            rhs=ones512[:, 0 : 512 - f0],
                            start=True,
                            stop=False,
                        )
                        pe_op((512 - f0) * PE_COL + MM_OVH)
                    nc.tensor.matmul(
                        lg[:, osl],
                        lhsT=kT[fs][hsl, bass.ds(kt * 128, 128)],
                        rhs=qT[fs][hsl, qsl],
                        start=allones,
                        stop=True,
                    )
                    pe_op((512 - f0) * PE_COL + MM_OVH)
                ex = expp.tile([128, 1024], BF16, tag="ex", name="ex")
                ready = clk["pe"] + SEM
                if all(f0 == 0 for f0 in f0s):
                    # one contiguous exp over the whole unit
                    nc.scalar.activation(
                        ex[:, 0 : 512 * len(kts)],
                        lg[:, 0 : 512 * len(kts)],
                        AF.Exp,
                        scale=SCALE,
                    )
                    clk["act"] = max(clk["act"], ready) + 512 * len(kts) * ACT_COL + ACT_OVH
                else:
                    # ragged trims: exp each kt's written range separately so
                    # no uninitialized psum gap is ever read
                    for i, f0 in enumerate(f0s):
                        rsl = bass.ds(i * 512 + f0, 512 - f0)
                        nc.scalar.activation(ex[:, rsl], lg[:, rsl], AF.Exp, scale=SCALE)
                        clk["act"] = max(clk["act"], ready) + (512 - f0) * ACT_COL + ACT_OVH
                done = clk["act"] + SEM
                for i, (kt, f0) in enumerate(zip(kts, f0s)):
                    if kt >= 4 * qb:  # diagonal tile: causal mask
                        w = 512 - f0
                        osl = bass.ds(i * 512 + f0, w)
                        nc.gpsimd.affine_select(
                            out=ex[:, osl],
                            in_=ex[:, osl],
                            compare_op=mybir.AluOpType.is_ge,
                            fill=0.0,
                            base=0,
                            channel_multiplier=-1,
                            pattern=[[1, w]],
                        )
                        done += w * ACT_COL + 150.0  # pool op in the exp->av chain
                exp_hist.append(done)
                return ex, done

            def _emit_one_av(unit):
                qb, h, kts, ex, exp_done = unit
                fs, hh = h // 2, h % 2
                # the av matmuls read v2 blocks for k-tiles <= kts[-1]
                drain_through(("v", kts[-1]))
                hsl = bass.ds(hh * 64, 64)
                qsl = bass.ds(qb * 512, 512)
                nkt = 4 * qb + 4
                if (qb, h) not in avs:
                    avs[(qb, h)] = psum.tile(
                        [128, 512], F32, tag="av", bufs=2, name="ps_av"
                    )
                av = avs[(qb, h)]
                for i, kt in enumerate(kts):
                    f0 = _f0(qb, kt)
                    nc.tensor.matmul(
                        av[:, bass.ds(f0, 512 - f0)],
                        lhsT=v2[:, kt * HPG + h, :],
                        rhs=ex[:, bass.ds(i * 512 + f0, 512 - f0)],
                        start=(kt == 0),
                        stop=(kt == nkt - 1),
                    )
                    pe_op((512 - f0) * PE_COL + MM_OVH, ready=(exp_done if i == 0 else 0.0))
                if kts[-1] == nkt - 1:  # head complete: normalize
                    # NOTE: custom-DVE ops (reciprocal_approx_fast) must read
                    # SBUF on hardware -- a direct PSUM input passes CoreSim
                    # but returns garbage on HW.  Hence the copy first.
                    den = recipp.tile([64, 512], F32, tag="den", name="den")
                    rf = recipp.tile([64, 512], F32, tag="rf", name="rf")
                    if qb == QB - 1 and h == HPG - 1:
                        # the very last normalize gates the final output
                        # projection tiles: emit it in 128-column pieces so
                        # oproj(st) can start as soon as its piece lands
                        for p in range(4):
                            psl = bass.ds(p * 128, 128)
                            qpl = bass.ds(qb * 512 + p * 128, 128)
                            nc.vector.tensor_copy(den[:, psl], av[64:128, psl])
                            nc.vector.reciprocal_approx_fast(rf[:, psl], den[:, psl])
                            nc.vector.tensor_mul(
                                avT[fs][hsl, qpl], av[0:64, psl], rf[:, psl]
                            )
                    else:
                        nc.vector.tensor_copy(den, av[64:128, :])
                        nc.vector.reciprocal_approx_fast(rf, den)
                        nc.vector.tensor_mul(avT[fs][hsl, qsl], av[0:64, :], rf)
                    del avs[(qb, h)]

            def flush_pending():
                while pending:
                    u = pending.pop(0)
                    drain_until_pe(u[4])
                    _emit_one_av(u)

            def emit_attention_head(qb, h):
                nkt = 4 * qb + 4
                for j in range(nkt // 2):
                    kts = [2 * j, 2 * j + 1]
                    # lg psum-bank WAR: the lg tag has 2 tiles; this unit's lg
                    # bank is free only once the exp two units back has read it
                    if len(exp_hist) >= 2:
                        drain_until_pe(exp_hist[-2] - SEM)
                    ex, exp_done = emit_unit_lgexp(qb, h, kts)
                    prev = pending[:]
                    del pending[:]
                    pending.append((qb, h, kts, ex, exp_done))
                    for u in prev:
                        drain_until_pe(u[4])
                        _emit_one_av(u)

            for qb in range(QB):
                for h in range(HPG):
                    with nc.named_scope(f"attn{qb}h{h}"):
                        emit_attention_head(qb, h)
                flush_pending()
                for st in range(4 * qb, 4 * qb + 4):
                    fillq.append(oproj_item(st, 0))
                    fillq.append(oproj_item(st, 1))
            with nc.named_scope("tailproj"):
                while fillq:
                    drain_one()

            if dbg:
                nc.sync.dma_start(out=dbg_v2.ap(), in_=v2)
                for i in range(2):
                    nc.sync.dma_start(out=dbg_avT.ap()[i], in_=avT[i])

    nc.compile()
    return nc


_CACHE: dict = {}


def kernel(
    x,
    padding_mask,
    wq_w,
    wq_b,
    wk_w,
    wk_b,
    wv_w,
    wv_b,
    wo_w,
    wo_b,
    **trace_kwargs,
):
    x = np.asarray(x, dtype=np.float32)
    padding_mask = np.asarray(padding_mask, dtype=np.float32)
    wq_w = np.asarray(wq_w, dtype=np.float32)
    wk_w = np.asarray(wk_w, dtype=np.float32)
    wv_w = np.asarray(wv_w, dtype=np.float32)
    wo_w = np.asarray(wo_w, dtype=np.float32)
    wq_b = np.asarray(wq_b, dtype=np.float32)
    wk_b = np.asarray(wk_b, dtype=np.float32)
    wv_b = np.asarray(wv_b, dtype=np.float32)
    wo_b = np.asarray(wo_b, dtype=np.float32)

    allones = bool(np.all(padding_mask == 1.0))
    if allones not in _CACHE:
        _CACHE[allones] = _build(allones)
    nc = _CACHE[allones]

    bf = ml_dtypes.bfloat16
    in_maps = []
    for c in range(8):
        b, g = c // 4, c % 4
        fsl = slice(g * F, (g + 1) * F)
        xTb = x[b].T.astype(bf)  # (1024, 2048)
        m = {
            # [4 sch, 128 p, KC, 512]: xT[d, s] with d = kc*128 + p
            "xT": np.ascontiguousarray(
                xTb.reshape(KC, 128, 4, 512).transpose(2, 1, 0, 3)
            ),
            "wq": np.ascontiguousarray(
                wq_w[:, fsl].astype(bf).reshape(KC, 128, F).transpose(1, 0, 2)
            ),
            "wk": np.ascontiguousarray(
                wk_w[:, fsl].astype(bf).reshape(KC, 128, F).transpose(1, 0, 2)
            ),
            "wv": np.ascontiguousarray(
                wv_w[:, fsl].astype(bf).reshape(KC, 128, F).transpose(1, 0, 2)
            ),
            "wo": np.ascontiguousarray(
                wo_w[fsl, :].astype(bf).reshape(2, 128, D).transpose(1, 0, 2)
            ),
            "bq": np.ascontiguousarray(wq_b[fsl].reshape(2, 128).T),
            "bk": np.ascontiguousarray(wk_b[fsl].reshape(2, 128).T),
        }
        if not allones:
            m["pad"] = ((padding_mask[b] - 1.0) * 8e9).reshape(1, S).astype(bf)
        in_maps.append(m)

    res = run_bass_kernel_spmd(nc, in_maps, core_ids=list(range(8)), **trace_kwargs)

    # host-side reduction over head groups (bf16 partials -> f32) + bias
    # correction
    correction = (wv_b @ wo_w + wo_b).astype(np.float32)
    out = np.empty((B, S, D), dtype=np.float32)
    for b in range(B):
        acc = res.results[4 * b]["out"].astype(np.float32)
        for g in range(1, G):
            acc += res.results[4 * b + g]["out"].astype(np.float32)
        out[b] = acc + correction
    kernel._last_results = res
    return out

